# revision 1
# baseline (speedup 1.0000x reference)
"""NT-Xent contrastive loss (forward) on 8 TRN2 NeuronCores via Bass/Tile.

Math: with h = concat(h_i, h_j) [N=8192, D=256], sim = (h @ h.T) / 0.5,
loss = mean_r( logsumexp_j(sim[r, j], j != r) - pos_r ), where
pos_r = sim[r, partner(r)] = 2 * h_i[q] . h_j[q].  The loss separates:
loss = (sum_r lse_r - sum_r pos_r) / N, and sum_r pos_r = 4 * sum(h_i * h_j).

Sharding: core c owns rows [1024c, 1024c + 1024).  Each core receives the
full transposed h, column-rotated by its row offset, so one SPMD program
serves all 8 cores: the self-similarity diagonal and the positive-pair
columns land at core-invariant positions.

Per core: the PE builds each 128-row block of sim in PSUM (bf16 operands,
fp32 accumulate) as four 1536-column chunks + two 1024-column chunks; the
diagonal is masked by accumulating I.T @ (-1e9 shifted-diag) as an extra
matmul; the scalar engine applies exp(2x - M_row) in place with a fused
row-sum (accum_out) on the 1536-chunks while the vector engine evaluates a
Schraudolph bit-trick exp (+-4%% per term, unbiased on average) on the
1024-chunks; the DVE also computes the positive-pair partial dots.  Each
core emits a [128, 52] tile of partial sums; the host finishes with
log/sum in float64.  M is a runtime input (per-row); if a row's exp-sum
under/overflows fp32, the host retries with a shifted M for those rows.
"""

import numpy as np
import ml_dtypes

B = 4096
D = 256
N = 2 * B            # 8192 rows/cols of sim
NCORES = 8
RPC = N // NCORES    # 1024 rows per core
KCH = D // 128       # 2 contraction chunks of 128
NJ = 4               # column chunks per row-block
CHUNK = N // NJ      # 2048 columns per chunk
NRB = RPC // 128     # 8 row-blocks of 128 per core
M_DEFAULT = 161.0    # logsumexp shift; safe while rowmax(2*h@h.T) in [M-70, M+79]
MASK_NEG = -1.0e9

# Schraudolph fast-exp constants (exp(y) ~= bitcast_f32(round(A*y + B)));
# B calibrated so the phase-averaged, exp-weighted relative error is ~1e-5
# (per-term max +-4%).  The DVE evaluates this for 1 of 4 column chunks per
# row-block, offloading a quarter of the exp work from the scalar engine;
# negative overflow saturates to INT_MIN = -0.0f which sums as zero.
EXP_A = float(2 ** 23 / np.log(2.0))
EXP_B = 1064865216.0

TRACE = False        # set True (e.g. from test.py) to request an NTFF trace
LAST_RESULTS = None  # BassKernelResults of the last run (for profiling)

_cache = {}


def _build():
    """Build the SPMD Bass/Tile program once per process."""
    if "nc" in _cache:
        return _cache["nc"]

    import concourse.tile as tile
    import concourse.mybir as mybir
    from concourse import bacc

    f32 = mybir.dt.float32
    bf16 = mybir.dt.bfloat16
    u32 = mybir.dt.uint32

    nc = bacc.Bacc("TRN2", target_bir_lowering=False, num_devices=NCORES)
    ht_dram = nc.dram_tensor("ht", [KCH, 128, N], bf16, kind="ExternalInput").ap()
    # eye[0][0] = I [128,128]; mask[v] [128,512] holds -1e9 at [p, 128v+p].
    # I.T @ mask[v] accumulated into a sim-block 512-slice masks its diagonal.
    eye_dram = nc.dram_tensor("eye", [1, 128, 128], bf16, kind="ExternalInput").ap()
    maskr_dram = nc.dram_tensor("maskr", [128, 4, 512], bf16, kind="ExternalInput").ap()
    bias_dram = nc.dram_tensor("biasm", [128, NRB], f32, kind="ExternalInput").ap()
    bias2_dram = nc.dram_tensor("bias2", [128, NRB], f32, kind="ExternalInput").ap()
    out_dram = nc.dram_tensor("out", [128, 52], f32, kind="ExternalOutput").ap()

    with tile.TileContext(nc) as tc:
        with (
            tc.tile_pool(name="hpool", bufs=1) as hpool,
            tc.tile_pool(name="small", bufs=1) as small,
            tc.tile_pool(name="scratch", bufs=1) as scratch,
            tc.tile_pool(name="ipool", bufs=3) as ipool,
            tc.tile_pool(name="psumA", bufs=2, space="PSUM") as psumA,
            tc.tile_pool(name="psumB", bufs=1, space="PSUM") as psumB,
        ):
            # Small constants go on the gpsimd (SWDGE) queue so they land
            # while the sync queue streams the big h.T chunks.
            eye_pos = small.tile([128, 128], bf16)
            nc.gpsimd.dma_start(out=eye_pos, in_=eye_dram[0])
            maskr_sb = small.tile([128, 4, 512], bf16)
            nc.gpsimd.dma_start(out=maskr_sb, in_=maskr_dram)
            bias_sb = small.tile([128, NRB], f32)
            nc.gpsimd.dma_start(out=bias_sb, in_=bias_dram)
            bias2_sb = small.tile([128, NRB], f32)
            nc.gpsimd.dma_start(out=bias2_sb, in_=bias2_dram)

            # Warm the ACT exp table (~2.7us load) during the DMA prologue so
            # the first real exp doesn't pay for it.
            warm_sb = small.tile([128, 1], f32)
            nc.scalar.activation(
                out=warm_sb, in_=bias_sb[:, 0:1],
                func=mybir.ActivationFunctionType.Exp, bias=0.0, scale=0.0,
            )

            # Warm the PE's HAM clock gate (cold = 1.2GHz for the first
            # ~3.4us of activity) with dummy matmuls on a memset tile while
            # the h.T DMAs are still in flight.
            wsrc = small.tile([128, 128], bf16)
            nc.vector.memset(wsrc, 0.0)
            wps = psumA.tile([128, 1536], f32, name="psA")
            for w in range(32):
                nc.tensor.matmul(
                    wps[:, (w % 3) * 512:(w % 3) * 512 + 128],
                    lhsT=wsrc, rhs=wsrc,
                    start=True, stop=True,
                )

            # h.T in SBUF on the sync HWDGE queue, in the order compute
            # consumes it.  Each DMA carries BOTH contraction halves of a
            # column range (tile layout [128, 2, width]) so the pipeline
            # never waits on a second transfer for the same columns.
            col_ranges = [(0, 1024), (1024, 2048), (2048, 3584),
                          (3584, 5120), (5120, 6656), (6656, 8192)]
            ht_tiles = []
            for di, (c0, c1) in enumerate(col_ranges):
                t = hpool.tile([128, KCH, c1 - c0], bf16, name=f"ht_{c0}")
                nc.sync.dma_start(
                    out=t,
                    in_=ht_dram[:, :, c0:c1].rearrange("k p c -> p k c"),
                )
                ht_tiles.append(t)

            def rhs_slice(k, c0, w=512):
                """[128, w] slice of rotated h.T at global column c0."""
                for (r0, r1), t in zip(col_ranges, ht_tiles):
                    if r0 <= c0 < r1:
                        assert c0 + w <= r1
                        return t[:, k, c0 - r0:c0 - r0 + w]
                raise AssertionError(c0)

            def lhsT_slice(k, rb):
                """[128, 128] row-block weights (columns rb*128..+128)."""
                return ht_tiles[0][:, k, rb * 128:(rb + 1) * 128]

            res_sb = small.tile([128, 52], f32)

            # Per row-block: 4 ACT chunks of 1536 columns (two 3-bank PSUM
            # slots) + 2 DVE fast-exp chunks of 1024 columns (one 2-bank
            # slot) = exactly 8 PSUM banks, with enough slot slack that the
            # PE never waits on a consumer.
            def emit_posdot():
                # Positive-pair partial dots: rotated columns [0,1024) are
                # this core's rows, [4096,5120) their partners.  Emitted
                # mid-stream so the DVE does it in slack, not on the tail.
                for k in range(KCH):
                    pp = scratch.tile([128, RPC], f32, name=f"ppscratch_{k}")
                    nc.vector.tensor_mul(pp, ht_tiles[0][:, k, :], ht_tiles[3][:, k, 512:512 + RPC])
                    nc.vector.reduce_sum(
                        res_sb[:, 48 + 2 * k:49 + 2 * k], pp, axis=mybir.AxisListType.X
                    )
                    nc.vector.memset(res_sb[:, 49 + 2 * k:50 + 2 * k], 0.0)

            def emit_B(rb, b):
                # DVE fast-exp chunk over columns [b*1024, b*1024+1024).
                # The diagonal (columns rb*128..+128) lies in b=0; mask it
                # with the I.T @ maskr accumulating matmul.
                psB = psumB.tile([128, 1024], f32, name="psB")
                cs0 = rb // 4
                for k in range(KCH):
                    lhsT = lhsT_slice(k, rb)
                    for cs in range(2):
                        nc.tensor.matmul(
                            psB[:, cs * 512:(cs + 1) * 512],
                            lhsT=lhsT,
                            rhs=rhs_slice(k, b * 1024 + cs * 512),
                            start=(k == 0),
                            stop=(k == KCH - 1) and not (b == 0 and cs == cs0),
                        )
                if b == 0:
                    nc.tensor.matmul(
                        psB[:, cs0 * 512:(cs0 + 1) * 512],
                        lhsT=eye_pos,
                        rhs=maskr_sb[:, rb % 4, :],
                        start=False,
                        stop=True,
                    )
                # bits = round(ps * 2A + (B - A*M_r)); sum the bitcast floats.
                ti = ipool.tile([128, 1024], u32, name="ti")
                nc.vector.tensor_scalar(
                    ti, psB, 2.0 * EXP_A, bias2_sb[:, rb:rb + 1],
                    mybir.AluOpType.mult, mybir.AluOpType.add,
                )
                nc.vector.reduce_sum(
                    res_sb[:, rb * 6 + 4 + b:rb * 6 + 5 + b],
                    ti.bitcast(f32),
                    axis=mybir.AxisListType.X,
                )

            def emit_A(rb, a):
                # ACT chunk over columns [2048 + a*1536, +1536).
                psA = psumA.tile([128, 1536], f32, name="psA")
                for k in range(KCH):
                    lhsT = lhsT_slice(k, rb)
                    for cs in range(3):
                        nc.tensor.matmul(
                            psA[:, cs * 512:(cs + 1) * 512],
                            lhsT=lhsT,
                            rhs=rhs_slice(k, 2048 + a * 1536 + cs * 512),
                            start=(k == 0),
                            stop=(k == KCH - 1),
                        )
                nc.scalar.activation(
                    out=psA,
                    in_=psA,
                    func=mybir.ActivationFunctionType.Exp,
                    bias=bias_sb[:, rb:rb + 1],
                    scale=2.0,
                    accum_out=res_sb[:, rb * 6 + a:rb * 6 + a + 1],
                )

            for rb in range(NRB):
                if rb == 5:
                    emit_posdot()
                # B (DVE) chunks interleaved between A (ACT) chunks so the
                # single B PSUM slot never stalls the PE, and each row-block
                # ends on an ACT chunk (short kernel tail).
                if rb == 0:
                    # First row-block consumes columns strictly in DMA
                    # arrival order.
                    for c in (("B", 0), ("B", 1), ("A", 0), ("A", 1),
                              ("A", 2), ("A", 3)):
                        (emit_B if c[0] == "B" else emit_A)(rb, c[1])
                else:
                    emit_B(rb, 0)
                    emit_A(rb, 0)
                    emit_B(rb, 1)
                    emit_A(rb, 1)
                    emit_A(rb, 2)
                    emit_A(rb, 3)

            # Ship rb0-6 partials while rb7 is still computing; only a
            # tiny transfer remains on the kernel tail.
            nc.sync.dma_start(out=out_dram[:, 0:42], in_=res_sb[:, 0:42])
            nc.sync.dma_start(out=out_dram[:, 42:52], in_=res_sb[:, 42:52])

    nc.compile()
    _cache["nc"] = nc
    return nc


def _make_static_inputs(h_i, h_j):
    """Per-core rotated h.T (bf16) plus the diag mask (shared)."""
    h = np.concatenate([np.asarray(h_i), np.asarray(h_j)], axis=0).astype(np.float32)
    hT = np.ascontiguousarray(h.T)  # [256, 8192]
    hts = []
    for c in range(NCORES):
        htc = np.roll(hT, -RPC * c, axis=1)
        hts.append(
            np.ascontiguousarray(htc.astype(ml_dtypes.bfloat16).reshape(KCH, 128, N))
        )
    eye = np.zeros((1, 128, 128), dtype=ml_dtypes.bfloat16)
    p = np.arange(128)
    eye[0, p, p] = 1.0
    maskr = np.zeros((128, 4, 512), dtype=ml_dtypes.bfloat16)
    for v in range(4):
        maskr[p, v, 128 * v + p] = MASK_NEG
    return hts, eye, maskr


def _axon_reset():
    """Recover the axon-tunneled NeuronCores if a previous process left them
    in an unrecoverable state."""
    try:
        import ctypes

        lib = ctypes.CDLL("/opt/axon/libaxon_pjrt.so")
        lib.axon_reset.restype = ctypes.c_int64
        return lib.axon_reset() == 0
    except Exception:
        return False


def _run(nc, hts, eye, maskr, M_per_core):
    global LAST_RESULTS
    from concourse import bass_utils

    in_maps = [
        {
            "ht": hts[c],
            "eye": eye,
            "maskr": maskr,
            "biasm": (-M_per_core[c]).astype(np.float32),
            "bias2": (EXP_B - EXP_A * M_per_core[c]).astype(np.float32),
        }
        for c in range(NCORES)
    ]
    try:
        results = bass_utils.run_bass_kernel_spmd(
            nc, in_maps, core_ids=list(range(NCORES)), trace=TRACE
        )
    except Exception:
        # A wedged accelerator (e.g. NRT_EXEC_UNIT_UNRECOVERABLE from an
        # earlier crashed process) survives process restarts; reset and retry.
        if not _axon_reset():
            raise
        results = bass_utils.run_bass_kernel_spmd(
            nc, in_maps, core_ids=list(range(NCORES)), trace=TRACE
        )
    LAST_RESULTS = results
    return results.results


def kernel(h_i, h_j):
    nc = _build()
    hts, eye, maskr = _make_static_inputs(h_i, h_j)

    # Per-core, per-row logsumexp shift M (as the activation bias -M).
    M = [np.full((128, NRB), M_DEFAULT, dtype=np.float64) for _ in range(NCORES)]

    lse = [np.full((128, NRB), np.nan) for _ in range(NCORES)]
    total_pd = 0.0

    for attempt in range(4):
        res = _run(nc, hts, eye, maskr, M)
        any_bad = False
        for c in range(NCORES):
            out = res[c]["out"].astype(np.float64)
            S = out[:, :48].reshape(128, NRB, 6).sum(axis=2)
            if attempt == 0:
                total_pd += out[:, 48:52].sum()
            good = np.isfinite(S) & (S > 0.0)
            upd = good & ~np.isfinite(lse[c])
            lse[c][upd] = M[c][upd] + np.log(S[upd])
            bad = ~np.isfinite(lse[c])
            if bad.any():
                any_bad = True
                # S == 0 -> M too high for those rows; S inf/nan -> too low.
                over = bad & ~np.isfinite(S)
                under = bad & ~over
                M[c][under] -= 75.0
                M[c][over] += 75.0
        if not any_bad:
            break

    total_lse = sum(l.sum() for l in lse)
    loss = (total_lse - 2.0 * total_pd) / float(N)
    return np.array(loss, dtype=np.float32)


if __name__ == "__main__":
    # Smoke test with random data (not the reference inputs).
    rng = np.random.default_rng(0)
    h_i = rng.standard_normal((B, D), dtype=np.float32)
    h_j = rng.standard_normal((B, D), dtype=np.float32)
    print("loss:", kernel(h_i, h_j))



# revision 2
# speedup vs baseline: 1.0059x; 1.0059x over previous
"""NT-Xent contrastive loss (forward) on 8 TRN2 NeuronCores via Bass/Tile.

v2: fp8(e4m3) DoubleRow matmuls + symmetric (half-matrix) evaluation.

Math: with h = concat(h_i, h_j) [N=8192, D=256], sim = (h @ h.T) / 0.5,
loss = mean_r( logsumexp_j(sim[r, j], j != r) - pos_r ).  pos is computed
exactly on the host (f64); the device produces per-row partial sums of
exp(sim - M) with a single global shift M = 161.

Symmetry: exp(sim[i,j] - M) == exp(sim[j,i] - M) for a global M, so each
128-row block only computes the cyclic column distances d = 0..32 (of 64
128-col blocks).  Row-sums over d = 0..32 come from ACT (exp + accum_out)
and DVE (Schraudolph bit-trick exp + fused tensor_tensor_reduce); the
remaining distances d = 33..63 for every row are recovered as COLUMN sums
of the d = 1..31 exp tiles, computed by the PE as ones-weighted matmuls
accumulating into a single PSUM bank laid out as [10, 512] f32.

Sharding: core c owns rows [1024c, 1024c + 1024).  Each core receives h.T
column-rotated by its row offset in fp8, pre-transposed to the DoubleRow
layout [128, 2, 5120] (only rotated columns [0, 5120) are ever touched:
row-block r uses columns [128r, 128r + 4224)).  One SPMD program serves
all 8 cores; the host reassembles row sums + column sums in f64, takes
logs, and averages.  Rows whose device sum is non-finite (never, for the
reference data) are recomputed exactly on the host.
"""

import os

import numpy as np
import ml_dtypes

DBG_NO_DR = os.environ.get("DBG_NO_DR", "") != ""
DBG_NO_COLSUM = os.environ.get("DBG_NO_COLSUM", "") != ""
DBG_NO_TTR = os.environ.get("DBG_NO_TTR", "") != ""

B = 4096
D = 256
N = 2 * B
NCORES = 8
RPC = N // NCORES     # 1024 rows per core
NRB = RPC // 128      # 8 row-blocks of 128 per core
M_DEFAULT = 161.0     # global logsumexp shift (rowmax(2 h.h) in [102, 240])
MASK_NEG = -1.0e9

SIMW = 33 * 128       # 4224: sim columns per row-block (distances 0..32)
CSW = 31 * 128        # 3968: colsum columns (distances 1..31)
RHSW = 7 * 128 + SIMW # 5120: rotated columns a core ever reads
HSPLIT = 4352         # first h.T tile: covers row-blocks 0 and 1 entirely
ACTW = 3072           # columns [0, 3072) of each row-block go to ACT
NSLOT = 6             # res slots per row-block (2 ACT + 3 DVE, 1 spare)

# Schraudolph fast-exp in bf16-bit space: exp(y) ~= bitcast_bf16(u16(A*y+B)).
EXP_A16 = float(2 ** 7 / np.log(2.0))
EXP_B16 = 16248.55

TRACE = False
LAST_RESULTS = None

_cache = {}


def _build():
    if "nc" in _cache:
        return _cache["nc"]

    import concourse.tile as tile
    import concourse.mybir as mybir
    from concourse import bacc

    f32 = mybir.dt.float32
    bf16 = mybir.dt.bfloat16
    f8 = mybir.dt.float8e4
    u16 = mybir.dt.uint16
    DR = mybir.MatmulPerfMode.DoubleRow
    ALU = mybir.AluOpType

    nc = bacc.Bacc("TRN2", target_bir_lowering=False, num_devices=NCORES)
    # ht's first 256 columns hold the DoubleRow identity and -240*I mask
    # planes so row 0's first matmuls ride the first DMA chunk.
    ht_dram = nc.dram_tensor("ht", [128, 2, 256 + RHSW], f8, kind="ExternalInput").ap()
    res_dram = nc.dram_tensor("res", [128, NRB * NSLOT], f32, kind="ExternalOutput").ap()
    csel_dram = nc.dram_tensor("csel", [128, 4, 4], bf16, kind="ExternalInput").ap()
    csum_dram = nc.dram_tensor("csum", [68, 512], f32, kind="ExternalOutput").ap()

    with tile.TileContext(nc) as tc:
        with (
            tc.tile_pool(name="hpool", bufs=1) as hpool,
            tc.tile_pool(name="small", bufs=1) as small,
            tc.tile_pool(name="exppool", bufs=2) as exppool,
            tc.tile_pool(name="tipool", bufs=2) as tipool,
            tc.tile_pool(name="psumA", bufs=2, space="PSUM") as psumA,
            tc.tile_pool(name="psumB", bufs=1, space="PSUM") as psumB,
            tc.tile_pool(name="psumC", bufs=1, space="PSUM") as psumC,
        ):
            # --- constants / warmup (overlap the big DMAs) ---

            ones_sb = small.tile([128, 512], bf16)
            nc.vector.memset(ones_sb, 1.0)
            zsel_sb = small.tile([128, 4], bf16)
            nc.vector.memset(zsel_sb, 0.0)
            csel_sb = small.tile([128, 4, 4], bf16)
            nc.gpsimd.dma_start(out=csel_sb, in_=csel_dram)
            wsrc = small.tile([128, 128], bf16)
            nc.vector.memset(wsrc, 0.0)
            res_sb = small.tile([128, NRB * NSLOT], f32)
            nc.vector.memset(res_sb, 0.0)
            csum_sb = small.tile([68, 512], f32)

            # ACT exp-table warm (one tiny activation loads the table).
            warm32 = small.tile([128, 1], f32)
            nc.vector.memset(warm32, 0.0)
            biasm = small.tile([128, 1], f32)
            nc.vector.memset(biasm, -M_DEFAULT)
            nc.scalar.activation(
                out=warm32, in_=warm32,
                func=mybir.ActivationFunctionType.Exp, bias=0.0, scale=0.0,
            )

            # h.T in fp8 DoubleRow layout, split so compute starts on the
            # first piece while the rest streams in.
            ht_tiles = []   # (rotated c0, c1, tile) -- tile col 0 = rot c0
            bounds = [-256, 512, 1536, 3072, RHSW]
            queues = [nc.sync, nc.sync, nc.gpsimd, nc.gpsimd]
            for bi in range(len(bounds) - 1):
                c0, c1 = bounds[bi], bounds[bi + 1]
                t = hpool.tile([128, 2, c1 - c0], f8, name=f"ht{c0}")
                queues[bi].dma_start(out=t, in_=ht_dram[:, :, c0 + 256:c1 + 256])
                ht_tiles.append((c0, c1, t))
            eye_sb = ht_tiles[0][2][:, :, 0:128]
            negd_sb = ht_tiles[0][2][:, :, 128:256]

            # PE HAM warm: ~32 dummy matmuls while the h.T DMA flies.
            wps = psumA.tile([128, 1536], f32, name="psA")
            for w in range(26):
                nc.tensor.matmul(
                    wps[:, (w % 3) * 512:(w % 3) * 512 + 128],
                    lhsT=wsrc, rhs=wsrc, start=True, stop=True,
                )

            # colsum accumulator: one PSUM bank; q-run r lands on partition
            # 32*(r%3) + r//3 so runs alternate PE column-groups and up to 3
            # colsum matmuls execute concurrently in the array.
            csum_ps = psumC.tile([128, 512], f32, name="psC")
            for g in range(3):
                nc.tensor.matmul(
                    csum_ps[32 * g:32 * g + 4, :], lhsT=zsel_sb, rhs=ones_sb,
                    start=True, stop=False, skip_group_check=True,
                    tile_position=(0, 32 * g),
                )

            def rhs_pieces(c0, w):
                """Split rotated column range [c0, c0+w) at tile seams."""
                out = []
                for t0, t1, t in ht_tiles:
                    if c0 < t1 and c0 + w > t0:
                        a, b = max(c0, t0), min(c0 + w, t1)
                        out.append((t[:, :, a - t0:b - t0], b - a))
                assert sum(pw for _, pw in out) == w
                return out

            def sim_chunk(ps, pofs, c0, w, start=True):
                """DR matmuls computing rotated cols [c0, c0+w) into ps[:, pofs:]."""
                pieces = rhs_pieces(c0, w)
                for i, (rhs, pw) in enumerate(pieces):
                    if DBG_NO_DR:
                        for k in range(2):
                            nc.tensor.matmul(
                                ps[:, pofs:pofs + pw],
                                lhsT=lhsT_r[:, k, :],
                                rhs=rhs[:, k, :],
                                start=start and (i == 0) and k == 0,
                                stop=(i == len(pieces) - 1) and k == 1,
                                skip_group_check=not start,
                            )
                    else:
                        nc.tensor.matmul(
                            ps[:, pofs:pofs + pw],
                            lhsT=lhsT_r,
                            rhs=rhs,
                            start=start and (i == 0),
                            stop=(i == len(pieces) - 1),
                            perf_mode=DR,
                            skip_group_check=not start,
                        )
                    pofs += pw

            def emit_colsums(r, exp_r, last, qlo=None, qhi=None):
                """PE column sums of exp_r's d=1..31 region into csum_ps.

                q-run p = q//4 lands on partition 32*(p%3) + p//3 at cols
                (q%4)*128; runs cycle the 3 PE column-groups.
                """
                qlo = r if qlo is None else qlo
                qhi = r + 30 if qhi is None else qhi
                for p in range(qlo // 4, qhi // 4 + 1):
                    q0 = max(4 * p, qlo)
                    q1 = min(4 * p + 3, qhi)
                    g, sub = p % 3, p // 3
                    nc.tensor.matmul(
                        csum_ps[32 * g:32 * g + 4,
                                128 * (q0 - 4 * p):128 * (q1 + 1 - 4 * p)],
                        lhsT=csel_sb[:, sub, :],
                        rhs=exp_r[:, 128 * (q0 + 1 - r):128 * (q1 + 2 - r)],
                        start=False,
                        stop=last and p == qhi // 4,
                        skip_group_check=True,
                        tile_position=(0, 32 * g),
                    )

            prev = None  # (r, exp_r)
            for r in range(NRB):
                base = 128 * r
                lhsT_r = rhs_pieces(base, 128)[0][0]
                exp_r = exppool.tile([128, SIMW], bf16, name="exp")

                def dve_sim(j, w):
                    psb = psumB.tile([128, 512], f32, name="psB")
                    sim_chunk(psb, 0, base + ACTW + 512 * j, w)
                    return psb

                def dve_ts(j, w, psb):
                    ti = tipool.tile([128, 512], u16, name="ti")
                    nc.vector.tensor_scalar(
                        ti[:, 0:w], psb[:, 0:w],
                        2.0 * EXP_A16, EXP_B16 - EXP_A16 * M_DEFAULT,
                        ALU.mult, ALU.add,
                    )
                    return ti

                def dve_post(j, w, ti):
                    nc.vector.tensor_scalar_add(
                        exp_r[:, ACTW + 512 * j:ACTW + 512 * j + w],
                        ti[:, 0:w].bitcast(bf16), 0.0,
                    )
                    nc.vector.reduce_sum(
                        res_sb[:, r * NSLOT + 2 + j:r * NSLOT + 3 + j],
                        ti[:, 0:w].bitcast(bf16),
                        axis=mybir.AxisListType.X,
                    )

                # Diagonal mask first (eye stationary), then sims share one
                # h stationary; the DVE-bank chunks go early so the vector
                # pipeline starts while the PE streams the ACT-bank chunks.
                psA0 = psumA.tile([128, 1536], f32, name="psA")
                nc.tensor.matmul(
                    psA0[:, 0:128], lhsT=eye_sb, rhs=negd_sb,
                    start=True, stop=False, perf_mode=DR,
                )
                if r == 0:
                    # DVE-bank columns arrive last on the DMA pipeline;
                    # stream the ACT banks first.
                    sim_chunk(psA0, 0, base, 512, start=False)
                    sim_chunk(psA0, 512, base + 512, 512)
                    sim_chunk(psA0, 1024, base + 1024, 512)
                    psA1 = psumA.tile([128, 1536], f32, name="psA")
                    for j in range(3):
                        sim_chunk(psA1, 512 * j, base + 1536 + 512 * j, 512)
                    psb0 = dve_sim(0, 512)
                    psb1 = dve_sim(1, 512)
                else:
                    sim_chunk(psA0, 0, base, 512, start=False)
                    psb0 = dve_sim(0, 512)
                    sim_chunk(psA0, 512, base + 512, 512)
                    sim_chunk(psA0, 1024, base + 1024, 512)
                    psb1 = dve_sim(1, 512)
                    psA1 = psumA.tile([128, 1536], f32, name="psA")
                    for j in range(3):
                        sim_chunk(psA1, 512 * j, base + 1536 + 512 * j, 512)

                ti0 = dve_ts(0, 512, psb0)
                ti1 = dve_ts(1, 512, psb1)
                dve_post(0, 512, ti0)

                nc.scalar.activation(
                    out=exp_r[:, 0:1536], in_=psA0,
                    func=mybir.ActivationFunctionType.Exp,
                    bias=biasm, scale=2.0,
                    accum_out=res_sb[:, r * NSLOT:r * NSLOT + 1],
                )

                psb2 = dve_sim(2, 128)
                if r == NRB - 1:
                    # Tail: the d=33..63 colsums only need the j0/j1 copies;
                    # emit them before everything else on the DVE.
                    nc.vector.tensor_scalar_add(
                        exp_r[:, ACTW + 512:ACTW + 1024],
                        ti1[:, 0:512].bitcast(bf16), 0.0,
                    )
                    nc.vector.reduce_sum(
                        res_sb[:, r * NSLOT + 3:r * NSLOT + 4],
                        ti1[:, 0:512].bitcast(bf16),
                        axis=mybir.AxisListType.X,
                    )
                    ti2 = dve_ts(2, 128, psb2)
                    dve_post(2, 128, ti2)
                else:
                    ti2 = dve_ts(2, 128, psb2)
                    dve_post(1, 512, ti1)
                    dve_post(2, 128, ti2)

                nc.scalar.activation(
                    out=exp_r[:, 1536:3072], in_=psA1,
                    func=mybir.ActivationFunctionType.Exp,
                    bias=biasm, scale=2.0,
                    accum_out=res_sb[:, r * NSLOT + 1:r * NSLOT + 2],
                )

                if prev is not None and not DBG_NO_COLSUM:
                    emit_colsums(prev[0], prev[1], last=False)
                if r == NRB - 1 and not DBG_NO_COLSUM:
                    emit_colsums(r, exp_r, last=False, qhi=r + 22)

                if r == 6:
                    # Ship finished row-block partials early.
                    nc.sync.dma_start(
                        out=res_dram[:, 0:5 * NSLOT], in_=res_sb[:, 0:5 * NSLOT]
                    )
                prev = (r, exp_r)

            if not DBG_NO_COLSUM:
                emit_colsums(prev[0], prev[1], last=True, qlo=prev[0] + 23)

            # Evacuate colsums PSUM -> SBUF -> DRAM; ship remaining res.
            nc.vector.tensor_scalar_add(csum_sb, csum_ps[0:68, :], 0.0)
            nc.scalar.dma_start(out=csum_dram, in_=csum_sb)
            nc.sync.dma_start(
                out=res_dram[:, 5 * NSLOT:], in_=res_sb[:, 5 * NSLOT:]
            )

    nc.compile()
    _cache["nc"] = nc
    return nc


def _make_inputs(h_i, h_j):
    h = np.concatenate([np.asarray(h_i), np.asarray(h_j)], axis=0).astype(np.float32)
    h8 = np.clip(h, -240.0, 240.0).astype(ml_dtypes.float8_e4m3)
    hT8 = np.ascontiguousarray(h8.T)  # [256, 8192]
    p = np.arange(128)
    head = np.zeros((128, 2, 256), dtype=ml_dtypes.float8_e4m3)
    head[p, :, p] = 1.0          # identity plane (DoubleRow: both k halves)
    head[p, :, 128 + p] = -240.0  # -480*I after the DR pair-sum
    hts = []
    for c in range(NCORES):
        rot = np.roll(hT8, -RPC * c, axis=1)[:, :RHSW]     # [256, 5120]
        rot = rot.reshape(2, 128, RHSW).transpose(1, 0, 2)
        hts.append(np.ascontiguousarray(np.concatenate([head, rot], axis=2)))
    csel = np.zeros((128, 4, 4), dtype=ml_dtypes.bfloat16)
    for q in range(4):
        csel[:, q, q] = 1.0
    return h, hts, csel


def _axon_reset():
    try:
        import ctypes

        lib = ctypes.CDLL("/opt/axon/libaxon_pjrt.so")
        lib.axon_reset.restype = ctypes.c_int64
        return lib.axon_reset() == 0
    except Exception:
        return False


def _run(nc, hts, csel):
    global LAST_RESULTS
    from concourse import bass_utils

    in_maps = [{"ht": hts[c], "csel": csel} for c in range(NCORES)]
    try:
        results = bass_utils.run_bass_kernel_spmd(
            nc, in_maps, core_ids=list(range(NCORES)), trace=TRACE
        )
    except Exception:
        if not _axon_reset():
            raise
        results = bass_utils.run_bass_kernel_spmd(
            nc, in_maps, core_ids=list(range(NCORES)), trace=TRACE
        )
    LAST_RESULTS = results
    return results.results


def kernel(h_i, h_j):
    nc = _build()
    h, hts, csel = _make_inputs(h_i, h_j)
    res = _run(nc, hts, csel)

    S = np.zeros(N, dtype=np.float64)
    for c in range(NCORES):
        r = res[c]["res"].astype(np.float64)          # [128, 48]
        part = r.reshape(128, NRB, NSLOT).sum(axis=2)  # [128, NRB]
        for rb in range(NRB):
            S[RPC * c + 128 * rb:RPC * c + 128 * (rb + 1)] += part[:, rb]
        cs = res[c]["csum"].astype(np.float64)         # [68, 512]
        for q in range(38):
            g = (128 * (q + 1) + RPC * c) % N
            p = q // 4
            S[g:g + 128] += cs[32 * (p % 3) + p // 3,
                               (q % 4) * 128:(q % 4) * 128 + 128]

    # pos on host, exact (f64)
    h_i64 = np.asarray(h_i, dtype=np.float64)
    h_j64 = np.asarray(h_j, dtype=np.float64)
    pos = 2.0 * (h_i64 * h_j64).sum(axis=1)
    pos_sum = 2.0 * pos.sum()

    bad = ~np.isfinite(S) | (S <= 0.0)
    lse = np.where(bad, 0.0, M_DEFAULT + np.log(np.where(bad, 1.0, S)))
    if bad.any():
        # exact host fallback for pathological rows
        h64 = np.concatenate([h_i64, h_j64], axis=0)
        for i in np.nonzero(bad)[0]:
            srow = 2.0 * (h64 @ h64[i])
            srow[i] = -np.inf
            m = srow.max()
            lse[i] = m + np.log(np.exp(srow - m).sum())

    loss = (lse.sum() - pos_sum) / float(N)
    return np.array(loss, dtype=np.float32)


if __name__ == "__main__":
    rng = np.random.default_rng(0)
    h_i = rng.standard_normal((B, D), dtype=np.float32)
    h_j = rng.standard_normal((B, D), dtype=np.float32)
    print("loss:", kernel(h_i, h_j))
